# revision 62
# baseline (speedup 1.0000x reference)
"""Tensor-parallel GQA attention forward for one TRN2 chip (8 NeuronCores).

Head-parallel strategy (v2):
  - host passes full xT (d-major, bf16) to every core; each core projects ONLY
    its own 4 q-heads + 1 kv-head with the weight tile stationary, producing
    qT/kT directly in attention layout (no receiver transposes, no projection
    collectives at all)
  - RoPE applied in transposed orientation (head_dim on partitions,
    de-interleaved [ev|od]); cos/sin tables arrive pre-transposed+replicated
  - chunk-pipelined schedule: attention for q-chunk c runs while the
    projection matmuls for chunk c+1 are interleaved into its exp-bound PE
    stalls (softmax exp on the scalar engine is the per-chunk bottleneck)
  - causal trimming: score matmuls + exp skip the below-diagonal-dead columns
    of diagonal k-tiles; probs outside get memset to 0 for the full-width PV
  - softmax denominators ride as a 65th..128th column of ones in the PV
    matmul; one AllToAll per head-pair flips attnT to sequence-sharded; each
    core then computes its 256-row slice of the output projection vs full wo
  - compute dtype bf16 (fp32 PSUM accumulation), output fp32
"""

import numpy as np

NC_CORES = 8
SEQ = 2048
DIM = 2048
HD = 64            # head dim
SC = SEQ // NC_CORES   # 256: sequence rows per core (output shard)
CH = 512           # q-chunk width for attention
NCH = SEQ // CH    # 4
KT = SEQ // 128    # 16 k-tiles
DT = DIM // 128    # 16 d-tiles
WCOLS = 384        # per-core weight cols: q pair0 (128) | q pair1 (128) | k 64 | v 64

_CACHE = {}


def _build_nc():
    import concourse.bass as bass
    import concourse.mybir as mybir
    import concourse.tile as tile
    from concourse import bacc
    from concourse.masks import make_identity

    BF = mybir.dt.bfloat16
    F32 = mybir.dt.float32
    MUL = mybir.AluOpType.mult
    ADD = mybir.AluOpType.add
    SUB = mybir.AluOpType.subtract

    nc = bacc.Bacc("TRN2", target_bir_lowering=False, debug=False,
                   num_devices=NC_CORES)

    # host pre-tiles everything into SBUF layout so DMAs are contiguous
    xT = nc.dram_tensor("xT", [NCH, 128, DT, CH], BF, kind="ExternalInput")
    wqkv = nc.dram_tensor("wqkv", [128, DT, WCOLS], BF, kind="ExternalInput")
    wo = nc.dram_tensor("wo", [128, DT, DIM], BF, kind="ExternalInput")
    cosr = nc.dram_tensor("cosr", [128, SEQ], BF, kind="ExternalInput")
    sinr = nc.dram_tensor("sinr", [128, SEQ], BF, kind="ExternalInput")
    out = nc.dram_tensor("out", [SC, DIM], F32, kind="ExternalOutput")

    groups = [list(range(NC_CORES))]

    with tile.TileContext(nc) as tc:
        # DRAM bounce buffers for the two attnT AllToAlls
        a2a_in0, _ = tc.tile([NC_CORES, 128, SC], BF,
                             space=bass.MemorySpace.DRAM, name="a2a_in0")
        a2a_out0, _ = tc.tile([NC_CORES, 128, SC], BF,
                              space=bass.MemorySpace.DRAM,
                              addr_space="Shared", name="a2a_out0")
        a2a_in1, _ = tc.tile([NC_CORES, 128, SC], BF,
                             space=bass.MemorySpace.DRAM, name="a2a_in1")
        a2a_out1, _ = tc.tile([NC_CORES, 128, SC], BF,
                              space=bass.MemorySpace.DRAM,
                              addr_space="Shared", name="a2a_out1")
        bar_in, _ = tc.tile([NC_CORES, 1, 256], BF,
                            space=bass.MemorySpace.DRAM, name="bar_in")
        bar_out, _ = tc.tile([NC_CORES, 1, 256], BF,
                             space=bass.MemorySpace.DRAM,
                             addr_space="Shared", name="bar_out")


        with tc.tile_pool(name="persist", bufs=1) as pp, \
             tc.tile_pool(name="work", bufs=2) as wp, \
             tc.tile_pool(name="psum", bufs=2, space="PSUM") as psp:

            ident = pp.tile([128, 128], BF, name="ident")
            make_identity(nc, ident[:])

            # triangle causal pattern, 1 where q-col >= k-row, for both heads
            patd = pp.tile([128, 2, 128], BF, name="patd")
            nc.gpsimd.memset(patd[:], 1.0)
            for h in range(2):
                nc.gpsimd.affine_select(
                    out=patd[:, h, :], in_=patd[:, h, :],
                    compare_op=mybir.AluOpType.is_ge, fill=0.0,
                    base=0, channel_multiplier=-1, pattern=[[1, 128]],
                )

            # prepay the exp ACT-table load (~2.7us) while DMAs stream
            warmup = pp.tile([1, 1], BF, name="warmup")
            nc.scalar.activation(warmup[:], ident[0:1, 0:1],
                                 mybir.ActivationFunctionType.Exp, scale=1.0)

            # startup DMA spread across the three queues so the first-needed
            # bytes land fastest: q weights on SP/ACT, kv+sin plus the first
            # xT half on the (fastest) gpsimd software queue
            wq_sb = [pp.tile([128, DT, 128], BF, name=f"wq{b}_sb")
                     for b in range(2)]
            wkv_sb = pp.tile([128, DT, 128], BF, name="wkv_sb")
            cos_sb = pp.tile([128, SEQ], BF, name="cos_sb")
            sin_sb = pp.tile([128, SEQ], BF, name="sin_sb")

            # xT as per-chunk half-tiles (dt 0-7 / 8-15) so matmuls start
            # when their half lands and two queues stream in parallel
            xt_tiles = [[wp.tile([128, 8, CH], BF, tag=f"xT{h}", bufs=2,
                                 name=f"xt{c}_{h}") for h in range(2)]
                        for c in range(NCH)]

            def issue_xt(c):
                nc.sync.dma_start(xt_tiles[c][0][:], xT[c, :, 0:8, :])
                nc.scalar.dma_start(xt_tiles[c][1][:], xT[c, :, 8:16, :])

            nc.gpsimd.dma_start(wq_sb[0][:], wqkv[:, :, 0:128])
            nc.gpsimd.dma_start(xt_tiles[0][0][:], xT[0, :, 0:8, :])
            nc.gpsimd.dma_start(wkv_sb[:], wqkv[:, :, 256:384])
            nc.gpsimd.dma_start(sin_sb[:], sinr[:])
            nc.scalar.dma_start(wq_sb[1][:], wqkv[:, :, 128:256])
            nc.scalar.dma_start(cos_sb[:], cosr[:])
            nc.sync.dma_start(xt_tiles[0][1][:], xT[0, :, 8:16, :])

            # wo groups are DMA'd later with data-dependency anchors so the
            # Tile scheduler cannot hoist them ahead of the xT stream
            wo_sb = pp.tile([128, DT, DIM], BF, name="wo_sb")

            # per-chunk kT / v tiles: separate tiles keep attention reads on
            # older chunks from false-depending on the newest chunk's writes
            kT_c = [pp.tile([128, CH], BF, name=f"kT{c}") for c in range(NCH)]
            v_c = [pp.tile([128, 4, 2 * HD], BF, name=f"v{c}")
                   for c in range(NCH)]
            for c in range(NCH):
                nc.gpsimd.memset(v_c[c][:, :, HD:2 * HD], 1.0)
            attnT = pp.tile([128, 2, SEQ], BF, name="attnT")

            qT_t = {}   # (chunk mod 2 handled by pool bufs) -> per-pair tiles

            def rope_apply(dst, src, nrows, sl):
                # dst = src*cos + rotate_half(src)*(+-sin); src is PSUM fp32
                # with rows in [ev(32)|od(32)] blocks; the ev-rows of sin_sb
                # carry a negated table so all tensor_tensor base partitions
                # align (NCC_IBIR297: 2-input SBUF ops need equal bases).
                rh = wp.tile([128, CH], F32, tag="rh", bufs=2, name="rh")
                for b in range(nrows // 64):
                    nc.vector.tensor_copy(rh[64 * b:64 * b + 32, :],
                                          src[64 * b + 32:64 * b + 64, :])
                    nc.vector.tensor_copy(rh[64 * b + 32:64 * b + 64, :],
                                          src[64 * b:64 * b + 32, :])
                t1 = wp.tile([128, CH], F32, tag="rp1", bufs=2, name="t1")
                nc.vector.tensor_tensor(t1[0:nrows, :], src[0:nrows, :],
                                        cos_sb[0:nrows, sl], MUL)
                nc.vector.tensor_tensor(rh[0:nrows, :], rh[0:nrows, :],
                                        sin_sb[0:nrows, sl], MUL)
                nc.vector.tensor_tensor(dst, t1[0:nrows, :], rh[0:nrows, :],
                                        ADD)

            def make_proj_tasks(c):
                """Projection of chunk c: (tasks, post) lists of closures.

                `tasks` are safe to interleave into an attention k-tile loop
                (each emits at most one PE op whose waits resolve on other
                engines); `post` (the v PE-transposes, which cycle the pv
                psum ring shared with open PV accumulators) may only run at
                the chunk drain.
                """
                xt = xt_tiles[c]
                sl = slice(CH * c, CH * c + CH)
                tasks = []
                pjq = psp.tile([128, 2, CH], F32, tag="pj", bufs=1,
                               name=f"pjq{c}")

                def q_mm(b, dt):
                    def f():
                        nc.tensor.matmul(
                            pjq[:, b, :], wq_sb[b][:, dt, :],
                            xt[dt // 8][:, dt % 8, :],
                            start=(dt == 0), stop=(dt == DT - 1))
                    return f

                def q_rope(b):
                    def f():
                        # bufs=4: pair-major attention needs every chunk's qT
                        # alive through the pair-1 pass
                        q = wp.tile([128, CH], BF, tag=f"qT{b}", bufs=4,
                                    name=f"q{b}_{c}")
                        qT_t[(c, b)] = q
                        rope_apply(q[:], pjq[:, b, :], 128, sl)
                    return f
                for b in range(2):
                    for dt in range(DT):
                        tasks.append(q_mm(b, dt))
                    tasks.append(q_rope(b))

                pjk = psp.tile([128, CH], F32, tag="pj", bufs=1,
                               name=f"pjk{c}")

                def kv_mm(dt):
                    def f():
                        nc.tensor.matmul(
                            pjk[:], wkv_sb[:, dt, :], xt[dt // 8][:, dt % 8, :],
                            start=(dt == 0), stop=(dt == DT - 1))
                    return f
                for dt in range(DT):
                    tasks.append(kv_mm(dt))

                def kv_fin():
                    # rope k (rows 0:64) into kT, duplicate to rows 64:128
                    rope_apply(kT_c[c][0:64, :], pjk[:], 64, sl)
                    nc.vector.tensor_copy(kT_c[c][64:128, :], kT_c[c][0:64, :])
                    # v: psum rows 64:128 -> staging -> PE transpose -> v_sb
                    vst = wp.tile([64, CH], BF, tag="vst", bufs=2, name="vst")
                    nc.vector.tensor_copy(vst[:], pjk[64:128, :])
                    qT_t[("vst", c)] = vst
                tasks.append(kv_fin)

                def v_tr(g):
                    def f():
                        vst = qT_t[("vst", c)]
                        tp = psp.tile([128, 128], BF, tag="pv", bufs=2,
                                      name="tp")
                        nc.tensor.transpose(
                            tp[:, 0:64], vst[:, 128 * g:128 * g + 128],
                            ident[0:64, 0:64])
                        nc.vector.tensor_copy(v_c[c][:, g, 0:HD],
                                              tp[:, 0:64])
                    return f
                post = [v_tr(g) for g in range(4)]
                return tasks, post

            def attention(c, p, filler):
                nkt = 4 * c + 4
                qTc = qT_t[(c, p)]
                qsl = slice(CH * c, CH * c + CH)
                pso0 = psp.tile([128, CH], F32, tag="pv", bufs=2, name="pso0")
                pso1 = psp.tile([128, CH], F32, tag="pv", bufs=2, name="pso1")
                pend = []   # (kt, ep, off) awaiting PV
                for kt in range(nkt):
                    kTk = kT_c[kt // 4]
                    ks = slice(128 * (kt % 4), 128 * (kt % 4) + 128)
                    dt_ = kt - 4 * c
                    off = 128 * dt_ if dt_ >= 0 else 0
                    sp = psp.tile([128, 2, CH], F32, tag="sp", bufs=2,
                                  name="sp")
                    nc.tensor.matmul(sp[:, 0, off:CH], kTk[0:64, ks],
                                     qTc[0:64, off:CH], start=True, stop=True)
                    nc.tensor.matmul(sp[:, 1, off:CH], kTk[64:128, ks],
                                     qTc[64:128, off:CH], start=True,
                                     stop=True)
                    ep = wp.tile([128, 2, CH], BF, tag="ep", bufs=3, name="ep")
                    nc.scalar.activation(ep[:, :, off:CH], sp[:, :, off:CH],
                                         mybir.ActivationFunctionType.Exp,
                                         scale=0.125)
                    if dt_ >= 0:
                        nc.vector.tensor_tensor(
                            ep[:, :, off:off + 128], ep[:, :, off:off + 128],
                            patd[:], MUL)
                    # drain previous k-tile's PV now (exp of this tile runs on
                    # ACT meanwhile), then interleave filler PE work.  PV is
                    # column-trimmed like the scores: columns below a diagonal
                    # tile's band take no contribution from it.
                    for (pkt, pep, poff) in pend:
                        vv = v_c[pkt // 4][:, pkt % 4, :]
                        nc.tensor.matmul(pso0[:, poff:CH], vv,
                                         pep[:, 0, poff:CH], start=(pkt == 0),
                                         stop=False)
                        nc.tensor.matmul(pso1[:, poff:CH], vv,
                                         pep[:, 1, poff:CH], start=(pkt == 0),
                                         stop=False)
                    pend = [(kt, ep, off)]
                    # delay pops so filler matmuls never reach the PE queue
                    # before their xT chunk has landed (FIFO stall hazard)
                    if nkt * p + kt >= 6:
                        for _ in range(3):
                            if filler:
                                filler.pop(0)()
                for (pkt, pep, poff) in pend:
                    vv = v_c[pkt // 4][:, pkt % 4, :]
                    nc.tensor.matmul(pso0[:, poff:CH], vv,
                                     pep[:, 0, poff:CH], start=(pkt == 0),
                                     stop=True)
                    nc.tensor.matmul(pso1[:, poff:CH], vv,
                                     pep[:, 1, poff:CH], start=(pkt == 0),
                                     stop=True)
                for h, pso in ((0, pso0), (1, pso1)):
                    bc = wp.tile([64, CH], F32, tag="bcast", bufs=2, name="bc")
                    nc.vector.tensor_copy(bc[:], pso[HD:2 * HD, :])
                    rc = wp.tile([64, CH], F32, tag="rcp", bufs=2, name="rc")
                    nc.vector.reciprocal_approx_fast(out=rc[:], in_=bc[:])
                    nc.vector.tensor_tensor(
                        attnT[64 * h:64 * h + 64, p, qsl],
                        pso[0:HD, :], rc[:], MUL)

            # ---------------- output projection helpers ----------------
            # per-src staging tiles so each outproj matmul depends only on
            # its own source's landing, not all eight
            a2a_sb0 = [pp.tile([128, SC], BF, name=f"a2a_sb0_{s}")
                       for s in range(NC_CORES)]
            a2a_sb1 = [pp.tile([128, SC], BF, name=f"a2a_sb1_{s}")
                       for s in range(NC_CORES)]
            partials = pp.tile([128, 2 * NCH, CH], BF, name="partials")
            evens = [2 * src for src in range(NC_CORES)]
            odds = [2 * src + 1 for src in range(NC_CORES)]
            chunks = [(qt, nch) for qt in range(2) for nch in range(NCH)]

            def op_mm(psf, qt, nsl, g, start, stop):
                sb = a2a_sb0 if g % 2 == 0 else a2a_sb1
                a_ap = sb[g // 2][:, 128 * qt:128 * qt + 128]
                nc.tensor.matmul(psf[:], a_ap, wo_sb[:, g, nsl],
                                 start=start, stop=stop)

            # ---------------- main pipeline (chunk-major) ----------------
            tasks, post = make_proj_tasks(0)
            for t in tasks + post:
                t()
            for c in range(NCH):
                if c + 1 < NCH:
                    issue_xt(c + 1)
                    filler, post = make_proj_tasks(c + 1)
                else:
                    filler, post = [], []
                if c in (2, 3):
                    # cheap mid-pipeline re-sync: absorbs accumulated
                    # inter-core skew while the PE has a deep backlog, so
                    # the big attnT AllToAlls later wait only on recent
                    # variance; the bar_in write anchored after the previous
                    # chunk's attention keeps the Tile scheduler from
                    # hoisting the collective to the head of the queue
                    nc.gpsimd.collective_compute(
                        "AllToAll", mybir.AluOpType.bypass,
                        replica_groups=groups, ins=[bar_in.opt()],
                        outs=[bar_out.opt()],
                    )
                attention(c, 0, filler)
                nc.sync.dma_start(bar_in[0, :, :], attnT[0:1, 0, 0:256])
                for dst in (2 * c, 2 * c + 1):
                    nc.sync.dma_start(a2a_in0[dst, :, :],
                                      attnT[:, 0, SC * dst:SC * dst + SC])
                # anchored wo prefetch: the 1-element write makes the DMA
                # wait until this point instead of competing with xT early;
                # scalar queue so the transfer never delays a2a_in writes
                nc.vector.tensor_copy(wo_sb[0:1, 4 * c, 0:1],
                                      attnT[0:1, 0, 0:1])
                nc.scalar.dma_start(wo_sb[:, 4 * c:4 * c + 4, :],
                                    wo[:, 4 * c:4 * c + 4, :])
                if c == NCH - 1:
                    nc.gpsimd.collective_compute(
                        "AllToAll", mybir.AluOpType.bypass,
                        replica_groups=groups, ins=[a2a_in0.opt()],
                        outs=[a2a_out0.opt()],
                    )
                    engs = [nc.sync, nc.scalar, nc.gpsimd]
                    for src in range(NC_CORES):
                        engs[src % 3].dma_start(a2a_sb0[src][:],
                                                a2a_out0[src, :, :])
                attention(c, 1, filler)
                for dst in (2 * c, 2 * c + 1):
                    nc.sync.dma_start(a2a_in1[dst, :, :],
                                      attnT[:, 1, SC * dst:SC * dst + SC])
                for t in filler + post:
                    t()
            nc.gpsimd.collective_compute(
                "AllToAll", mybir.AluOpType.bypass,
                replica_groups=groups, ins=[a2a_in1.opt()],
                outs=[a2a_out1.opt()],
            )
            for src in range(NC_CORES):
                eng = nc.sync if src % 2 == 0 else nc.scalar
                eng.dma_start(a2a_sb1[src][:], a2a_out1[src, :, :])

            for i8, (qt, nch) in enumerate(chunks):
                psf = psp.tile([128, CH], F32, tag="sp", bufs=2, name="psfE")
                nsl = slice(CH * nch, CH * nch + CH)
                for i, g in enumerate(evens):
                    op_mm(psf, qt, nsl, g, i == 0, i == NC_CORES - 1)
                nc.vector.tensor_copy(partials[:, i8, :], psf[:])
            for i8, (qt, nch) in enumerate(chunks):
                psf = psp.tile([128, CH], F32, tag="sp", bufs=2, name="psfO")
                nsl = slice(CH * nch, CH * nch + CH)
                for i, g in enumerate(odds):
                    op_mm(psf, qt, nsl, g, i == 0, i == NC_CORES - 1)
                osb = wp.tile([128, CH], F32, tag="osb", bufs=2, name="osb")
                nc.vector.tensor_tensor(osb[:], psf[:], partials[:, i8, :],
                                        ADD)
                nc.sync.dma_start(out[128 * qt:128 * qt + 128, nsl], osb[:])

    nc.finalize()
    return nc


def _get_nc():
    if "nc" not in _CACHE:
        _CACHE["nc"] = _build_nc()
    return _CACHE["nc"]


_PERM = np.concatenate([np.arange(0, HD, 2), np.arange(1, HD, 2)])  # de-interleave


def _shard(inputs):
    import ml_dtypes
    BF = ml_dtypes.bfloat16
    x = np.asarray(inputs["x"][0], dtype=np.float32)                 # [S, D]
    # [D, S] -> chunk/partition tiling [NCH, 128, DT, CH] (contiguous DMAs)
    xT = np.ascontiguousarray(
        x.T.astype(BF).reshape(DT, 128, NCH, CH).transpose(2, 1, 0, 3))
    wq = np.asarray(inputs["wq"], dtype=np.float32)
    wk = np.asarray(inputs["wk"], dtype=np.float32)
    wv = np.asarray(inputs["wv"], dtype=np.float32)
    wo = np.ascontiguousarray(
        np.asarray(inputs["wo"]).astype(BF)
        .reshape(DT, 128, DIM).transpose(1, 0, 2))                   # [128,DT,D]
    cos = np.asarray(inputs["freqs_cos"], dtype=np.float32)          # [S, 32]
    sin = np.asarray(inputs["freqs_sin"], dtype=np.float32)
    cosr = np.ascontiguousarray(np.tile(cos.T, (4, 1)).astype(BF))   # [128, S]
    # ev-rows get -sin so rotate_half(x)*sinr lands with the right signs
    sinr = np.ascontiguousarray(
        np.concatenate([-sin.T, sin.T, -sin.T, sin.T], axis=0).astype(BF))
    wq_p = wq.reshape(DIM, 32, HD)[:, :, _PERM]                      # [D,32,64]
    wk_p = wk.reshape(DIM, 8, HD)[:, :, _PERM]
    in_maps = []
    for c in range(NC_CORES):
        q0 = wq_p[:, 4 * c:4 * c + 2, :].reshape(DIM, 128)
        q1 = wq_p[:, 4 * c + 2:4 * c + 4, :].reshape(DIM, 128)
        kc = wk_p[:, c, :]
        vc = wv[:, HD * c:HD * c + HD]
        wqkv = np.ascontiguousarray(
            np.concatenate([q0, q1, kc, vc], axis=1).astype(BF)
            .reshape(DT, 128, WCOLS).transpose(1, 0, 2))             # [128,DT,W]
        in_maps.append({
            "xT": xT,
            "wqkv": wqkv,
            "wo": wo,
            "cosr": cosr,
            "sinr": sinr,
        })
    return in_maps


def kernel(**inputs):
    from concourse.bass_utils import run_bass_kernel_spmd

    nc = _get_nc()
    in_maps = _shard(inputs)
    res = run_bass_kernel_spmd(nc, in_maps, core_ids=list(range(NC_CORES)))
    out = np.concatenate([res.results[c]["out"] for c in range(NC_CORES)],
                         axis=0)
    return out[None].astype(np.float32)


# revision 65
# speedup vs baseline: 1.0322x; 1.0322x over previous
"""Tensor-parallel GQA attention forward for one TRN2 chip (8 NeuronCores).

Strategy (8-way tensor parallel over heads):
  - each core owns 4 q-heads + 1 kv-head (wq/wk/wv column-sharded, host side)
  - x is transposed on-device: each core PE-transposes its 256-row slice of x
    (cast to bf16) and an AllGather assembles the full xT on every core
  - projections produce qT/kT (head_dim on partitions) and v (natural layout)
    directly in the layouts the attention matmuls want; RoPE is applied in a
    de-interleaved head-dim ordering (dot products are permutation invariant)
  - scores are computed transposed (S^T[k, q]) so exp runs straight out of
    PSUM; softmax denominators come for free as a 65th column of ones in the
    PV matmul; causal masking = skipping k-tiles above the diagonal plus a
    0/1 pattern multiply on the 4 diagonal-band tiles per chunk
  - an AllToAll flips head-sharded attnT to sequence-sharded, each core then
    computes its 256-row slice of the output projection against full wo
  - compute dtype bf16 (fp32 PSUM accumulation), output fp32
"""

import numpy as np

NC_CORES = 8
SEQ = 2048
DIM = 2048
HD = 64            # head dim
LHEADS = 4         # q heads per core
SC = SEQ // NC_CORES   # 256: sequence rows per core (transpose shard / output shard)
CH = 512           # q-chunk width for attention
NCH = SEQ // CH    # 4
KT = SEQ // 128    # 16 k-tiles
DT = DIM // 128    # 16 d-tiles

_CACHE = {}


def _build_nc():
    import concourse.bass as bass
    import concourse.mybir as mybir
    import concourse.tile as tile
    from concourse import bacc
    from concourse.masks import make_identity

    BF = mybir.dt.bfloat16
    F32 = mybir.dt.float32
    MUL = mybir.AluOpType.mult
    ADD = mybir.AluOpType.add
    SUB = mybir.AluOpType.subtract

    nc = bacc.Bacc("TRN2", target_bir_lowering=False, debug=False,
                   num_devices=NC_CORES)

    # ---- external I/O (per-core shards) ----
    # W_all columns: [q-pair0: 8x128 | q-pair1: 8x128 | k: 8x64 | v: 8x64]
    x_sl = nc.dram_tensor("x_sl", [SC, DIM], F32, kind="ExternalInput")
    w_all = nc.dram_tensor("w_all", [DIM, DIM + 2 * 512], BF, kind="ExternalInput")
    wo = nc.dram_tensor("wo", [DIM, DIM], BF, kind="ExternalInput")
    cosR = nc.dram_tensor("cosR", [SC, 32], F32, kind="ExternalInput")
    sinR = nc.dram_tensor("sinR", [SC, 32], F32, kind="ExternalInput")
    out = nc.dram_tensor("out", [SC, DIM], F32, kind="ExternalOutput")

    groups = [list(range(NC_CORES))]
    WCOLS = DIM + 1024          # 3072
    NCH_W = WCOLS // CH         # 6 projection column chunks

    with tile.TileContext(nc) as tc:
        # DRAM bounce buffers for collectives
        apkv_in, _ = tc.tile([NC_CORES, SC, 128], BF, space=bass.MemorySpace.DRAM,
                             name="apkv_in")
        apkv_out, _ = tc.tile([NC_CORES, SC, 128], BF, space=bass.MemorySpace.DRAM,
                              addr_space="Shared", name="apkv_out")
        apq0_in, _ = tc.tile([NC_CORES, SC, 128], BF, space=bass.MemorySpace.DRAM,
                             name="apq0_in")
        apq0_out, _ = tc.tile([NC_CORES, SC, 128], BF, space=bass.MemorySpace.DRAM,
                              addr_space="Shared", name="apq0_out")
        apq1_in, _ = tc.tile([NC_CORES, SC, 128], BF, space=bass.MemorySpace.DRAM,
                             name="apq1_in")
        apq1_out, _ = tc.tile([NC_CORES, SC, 128], BF, space=bass.MemorySpace.DRAM,
                              addr_space="Shared", name="apq1_out")
        a2a_in0, _ = tc.tile([NC_CORES, 128, SC], BF,
                             space=bass.MemorySpace.DRAM, name="a2a_in0")
        a2a_out0, _ = tc.tile([NC_CORES, 128, SC], BF,
                              space=bass.MemorySpace.DRAM,
                              addr_space="Shared", name="a2a_out0")
        a2a_in1, _ = tc.tile([NC_CORES, 128, SC], BF,
                             space=bass.MemorySpace.DRAM, name="a2a_in1")
        a2a_out1, _ = tc.tile([NC_CORES, 128, SC], BF,
                              space=bass.MemorySpace.DRAM,
                              addr_space="Shared", name="a2a_out1")

        with tc.tile_pool(name="persist", bufs=1) as pp, \
             tc.tile_pool(name="wstream", bufs=2) as wsp, \
             tc.tile_pool(name="work", bufs=2) as wp, \
             tc.tile_pool(name="psum", bufs=2, space="PSUM") as psp:

            # ---------------- local transpose of own x slice ----------------
            ident = pp.tile([128, 128], BF, name="ident")
            make_identity(nc, ident[:])

            # prepay the exp ACT-table load (~2.7us) while DMAs stream
            warmup = pp.tile([1, 1], BF, name="warmup")
            nc.scalar.activation(warmup[:], ident[0:1, 0:1],
                                 mybir.ActivationFunctionType.Exp, scale=1.0)

            xsl_bf = pp.tile([128, 2, DIM], BF, name="xsl_bf")
            for pt in range(2):
                nc.gpsimd.dma_start(
                    xsl_bf[:, pt, :], x_sl[128 * pt:128 * pt + 128, :])

            xTc = pp.tile([128, DT, SC], BF, name="xTc")
            for pt in range(2):
                for j in range(DT):
                    trp = psp.tile([128, 128], BF, tag="ps", bufs=4, name="trp")
                    nc.tensor.transpose(trp[:], xsl_bf[:, pt, 128 * j:128 * j + 128],
                                        ident[:])
                    nc.vector.tensor_copy(xTc[:, j, 128 * pt:128 * pt + 128], trp[:])

            # rope tables, replicated across the 40 roped heads (32 q + 8 k),
            # per local 128-row s-tile
            cosR_sb = pp.tile([128, 2, 32], BF, name="cosR_sb")
            sinR_sb = pp.tile([128, 2, 32], BF, name="sinR_sb")
            nc.gpsimd.dma_start(cosR_sb[:], cosR[:].rearrange("(t p) f -> p t f", p=128))
            nc.gpsimd.dma_start(sinR_sb[:], sinR[:].rearrange("(t p) f -> p t f", p=128))
            cos_rep = pp.tile([128, 2, 8, 32], BF, name="cos_rep")
            sin_rep = pp.tile([128, 2, 8, 32], BF, name="sin_rep")
            for st in range(2):
                for h in range(8):
                    nc.vector.tensor_copy(cos_rep[:, st, h, :], cosR_sb[:, st, :])
                    nc.vector.tensor_copy(sin_rep[:, st, h, :], sinR_sb[:, st, :])

            # ---------------- seq-sharded projections (all heads, own 256 s) ----
            # W chunk order: k, v first (their A2A overlaps the q projections),
            # then q-pair0, then q-pair1; each section's AllToAll is issued as
            # soon as its columns are projected + roped.
            proj = pp.tile([128, 2, WCOLS], BF, name="proj")

            def proj_chunk(ch):
                wt = wsp.tile([128, DT, CH], BF, tag="wt", bufs=2, name="wt")
                for hf in range(2):
                    nc.sync.dma_start(
                        wt[:, 8 * hf:8 * hf + 8, :],
                        w_all[1024 * hf:1024 * hf + 1024, CH * ch:CH * ch + CH]
                        .rearrange("(t p) m -> p t m", p=128))
                for st in range(2):
                    psq = psp.tile([128, CH], F32, tag="ps", bufs=4, name="psq")
                    for dt in range(DT):
                        nc.tensor.matmul(
                            psq[:], xTc[:, dt, 128 * st:128 * st + 128],
                            wt[:, dt, :],
                            start=(dt == 0), stop=(dt == DT - 1))
                    if ch < 5:   # q and k columns get RoPE (8 head-pairs/chunk)
                        nh = 8
                        pv = psq[:].rearrange("p (h x) -> p h x", x=32)
                        ta = wp.tile([128, 8, 32], F32, tag="ropeA", bufs=2, name="ta")
                        tb = wp.tile([128, 8, 32], F32, tag="ropeB", bufs=2, name="tb")
                        dstv = proj[:, st, CH * ch:CH * ch + CH].rearrange(
                            "p (h x) -> p h x", x=32)
                        crep = cos_rep[:, st, 0:nh, :]
                        srep = sin_rep[:, st, 0:nh, :]
                        qr = pv[:, 0:2 * nh:2, :]
                        qi = pv[:, 1:2 * nh:2, :]
                        nc.vector.tensor_tensor(ta[:, 0:nh, :], qr, crep, MUL)
                        nc.vector.tensor_tensor(tb[:, 0:nh, :], qi, srep, MUL)
                        nc.vector.tensor_tensor(dstv[:, 0:2 * nh:2, :],
                                                ta[:, 0:nh, :], tb[:, 0:nh, :], SUB)
                        nc.vector.tensor_tensor(ta[:, 0:nh, :], qr, srep, MUL)
                        nc.vector.tensor_tensor(tb[:, 0:nh, :], qi, crep, MUL)
                        nc.vector.tensor_tensor(dstv[:, 1:2 * nh:2, :],
                                                ta[:, 0:nh, :], tb[:, 0:nh, :], ADD)
                    else:
                        nc.vector.tensor_copy(proj[:, st, CH * ch:CH * ch + CH],
                                              psq[:])

            # --- kv section (stores issued per chunk for an earlier trigger) ---
            proj_chunk(4)
            for dst in range(NC_CORES):
                nc.gpsimd.dma_start(
                    apkv_in[dst, :, 0:64].rearrange("(t p) m -> p t m", p=128),
                    proj[:, :, 2048 + 64 * dst:2048 + 64 * dst + 64])
            proj_chunk(5)
            for dst in range(NC_CORES):
                nc.gpsimd.dma_start(
                    apkv_in[dst, :, 64:128].rearrange("(t p) m -> p t m", p=128),
                    proj[:, :, 2560 + 64 * dst:2560 + 64 * dst + 64])
            nc.gpsimd.collective_compute(
                "AllToAll", mybir.AluOpType.bypass,
                replica_groups=groups, ins=[apkv_in.opt()], outs=[apkv_out.opt()],
            )
            # --- q pair 0 ---
            for ch in (0, 1):
                proj_chunk(ch)
                for dst in range(4 * ch, 4 * ch + 4):
                    nc.gpsimd.dma_start(
                        apq0_in[dst, :, :].rearrange("(t p) m -> p t m", p=128),
                        proj[:, :, 128 * dst:128 * dst + 128])
            nc.gpsimd.collective_compute(
                "AllToAll", mybir.AluOpType.bypass,
                replica_groups=groups, ins=[apq0_in.opt()], outs=[apq0_out.opt()],
            )
            # --- q pair 1 ---
            for ch in (2, 3):
                proj_chunk(ch)
                for dst in range(4 * (ch - 2), 4 * (ch - 2) + 4):
                    nc.gpsimd.dma_start(
                        apq1_in[dst, :, :].rearrange("(t p) m -> p t m", p=128),
                        proj[:, :, 1024 + 128 * dst:1024 + 128 * dst + 128])
            nc.gpsimd.collective_compute(
                "AllToAll", mybir.AluOpType.bypass,
                replica_groups=groups, ins=[apq1_in.opt()], outs=[apq1_out.opt()],
            )

            # ---------------- receiver: build kT / v, then qT per pair ----------
            qT_t = [[pp.tile([128, CH], BF, name=f"qT{p}_{j}")
                     for j in range(NCH)] for p in range(2)]
            kT = pp.tile([128, SEQ], BF, name="kT")
            v_sb = pp.tile([128, KT, 2 * HD], BF, name="v_sb")
            nc.gpsimd.memset(v_sb[:, :, HD:2 * HD], 1.0)

            stage_k = pp.tile([128, KT, 64], BF, name="stage_k")
            for src in range(NC_CORES):
                nc.sync.dma_start(
                    stage_k[:, 2 * src:2 * src + 2, :],
                    apkv_out[src, :, 0:64].rearrange("(t p) m -> p t m", p=128))
                nc.sync.dma_start(
                    v_sb[:, 2 * src:2 * src + 2, 0:HD],
                    apkv_out[src, :, 64:128].rearrange("(t p) m -> p t m", p=128))
            for g in range(KT):
                tk = psp.tile([64, 128], BF, tag="ps", bufs=4, name="tk")
                nc.tensor.transpose(tk[:], stage_k[:, g, :], ident[:])
                nc.vector.tensor_copy(kT[0:64, 128 * g:128 * g + 128], tk[:])
            nc.vector.tensor_copy(kT[64:128, :], kT[0:64, :])

            stage_q = pp.tile([128, 2, KT, 128], BF, name="stage_q")

            def build_qT(pair):
                apq_out = apq0_out if pair == 0 else apq1_out
                for src in range(NC_CORES):
                    nc.sync.dma_start(
                        stage_q[:, pair, 2 * src:2 * src + 2, :],
                        apq_out[src, :, :].rearrange("(t p) m -> p t m", p=128))
                    for st in range(2):
                        g = 2 * src + st
                        tq = psp.tile([128, 128], BF, tag="ps", bufs=4, name="tq")
                        tq_in = stage_q[:, pair, g, :]
                        nc.tensor.transpose(tq[:], tq_in, ident[:])
                        nc.vector.tensor_copy(
                            qT_t[pair][g // 4][:, 128 * (g % 4):128 * (g % 4) + 128],
                            tq[:])

            build_qT(0)

            # causal pattern: one triangle block (q-col >= k-row), both heads
            patd = pp.tile([128, 2, 128], BF, name="patd")
            nc.gpsimd.memset(patd[:], 1.0)
            for half in range(2):
                nc.gpsimd.affine_select(
                    out=patd[:, half, :], in_=patd[:, half, :],
                    compare_op=mybir.AluOpType.is_ge, fill=0.0,
                    base=0, channel_multiplier=-1, pattern=[[1, 128]],
                )

            # ---------------- attention ----------------
            attnT = pp.tile([128, 2, SEQ], BF, name="attnT")

            def attention(pair, j):
                # software-pipelined k-tile loop: scores(t+1) issue while
                # exp(t) runs on ACT, PV(t) follows; scores/exp/PV are all
                # column-trimmed to the causal region of diagonal k-tiles
                nkt = 4 * j + 4
                pso0 = psp.tile([2 * HD, CH], F32, tag="ps", bufs=4, name="pso0")
                pso1 = psp.tile([2 * HD, CH], F32, tag="ps", bufs=4, name="pso1")
                qsl = slice(CH * j, CH * j + CH)
                qTc = qT_t[pair][j]
                pend = []
                for kt in range(nkt):
                    ks = slice(128 * kt, 128 * kt + 128)
                    dt_ = kt - 4 * j
                    off = 128 * dt_ if dt_ >= 0 else 0
                    sp = psp.tile([128, 2, CH], F32, tag="spair", bufs=2,
                                  name="sp")
                    nc.tensor.matmul(sp[:, 0, off:CH], kT[0:64, ks],
                                     qTc[0:64, off:CH], start=True, stop=True)
                    nc.tensor.matmul(sp[:, 1, off:CH], kT[64:128, ks],
                                     qTc[64:128, off:CH], start=True,
                                     stop=True)
                    ep = wp.tile([128, 2, CH], BF, tag="exps", bufs=4,
                                 name="ep")
                    nc.scalar.activation(ep[:, :, off:CH], sp[:, :, off:CH],
                                         mybir.ActivationFunctionType.Exp,
                                         scale=0.125)
                    if dt_ >= 0:
                        nc.vector.tensor_tensor(
                            ep[:, :, off:off + 128], ep[:, :, off:off + 128],
                            patd[:], MUL)
                    for (pkt, pep, poff) in pend:
                        nc.tensor.matmul(pso0[:, poff:CH], v_sb[:, pkt, :],
                                         pep[:, 0, poff:CH],
                                         start=(pkt == 0), stop=False)
                        nc.tensor.matmul(pso1[:, poff:CH], v_sb[:, pkt, :],
                                         pep[:, 1, poff:CH],
                                         start=(pkt == 0), stop=False)
                    pend = [(kt, ep, off)]
                for (pkt, pep, poff) in pend:
                    nc.tensor.matmul(pso0[:, poff:CH], v_sb[:, pkt, :],
                                     pep[:, 0, poff:CH], start=(pkt == 0),
                                     stop=True)
                    nc.tensor.matmul(pso1[:, poff:CH], v_sb[:, pkt, :],
                                     pep[:, 1, poff:CH], start=(pkt == 0),
                                     stop=True)
                for h, pso in ((0, pso0), (1, pso1)):
                    bc = wp.tile([64, CH], F32, tag="bcast", bufs=2, name="bc")
                    nc.vector.tensor_copy(bc[:], pso[HD:2 * HD, :])
                    rc = wp.tile([64, CH], F32, tag="rcp", bufs=2, name="rc")
                    nc.vector.reciprocal_approx_fast(out=rc[:], in_=bc[:])
                    nc.vector.tensor_tensor(
                        attnT[64 * h:64 * h + 64, pair, qsl],
                        pso[0:HD, :], rc[:], MUL)

            woA = pp.tile([128, DT // 2, DIM], BF, name="woA")
            woB = pp.tile([128, DT // 2, DIM], BF, name="woB")
            for j in range(NCH):
                attention(0, j)
                if j == 1:
                    build_qT(1)   # overlaps remaining pair-0 attention
                for dst in (2 * j, 2 * j + 1):
                    nc.gpsimd.dma_start(a2a_in0[dst, :, :],
                                        attnT[:, 0, SC * dst:SC * dst + SC])
                # anchored wo prefetch (the scheduler hoists dep-free DMAs)
                nc.vector.tensor_copy(woA[0:1, 2 * j, 0:1],
                                      attnT[0:1, 0, CH * j:CH * j + 1])
                nc.sync.dma_start(
                    woA[:, 2 * j:2 * j + 2, :],
                    wo[256 * j:256 * j + 256, :].rearrange("(t p) n -> p t n",
                                                           p=128))
            nc.gpsimd.collective_compute(
                "AllToAll", mybir.AluOpType.bypass,
                replica_groups=groups, ins=[a2a_in0.opt()], outs=[a2a_out0.opt()],
            )
            a2a_sb0 = pp.tile([128, NC_CORES, SC], BF, name="a2a_sb0")
            a2a_sb1 = pp.tile([128, NC_CORES, SC], BF, name="a2a_sb1")
            for src in range(NC_CORES):
                nc.sync.dma_start(a2a_sb0[:, src, :], a2a_out0[src, :, :])
            for j in range(NCH):
                attention(1, j)
                for dst in (2 * j, 2 * j + 1):
                    nc.gpsimd.dma_start(a2a_in1[dst, :, :],
                                        attnT[:, 1, SC * dst:SC * dst + SC])
                if j < 2:
                    nc.vector.tensor_copy(woB[0:1, 4 * j, 0:1],
                                          attnT[0:1, 1, CH * j:CH * j + 1])
                    nc.sync.dma_start(
                        woB[:, 4 * j:4 * j + 4, :],
                        wo[1024 + 512 * j:1024 + 512 * j + 512, :]
                        .rearrange("(t p) n -> p t n", p=128))

            # ---------------- final A2A + output projection ----------------
            nc.gpsimd.collective_compute(
                "AllToAll", mybir.AluOpType.bypass,
                replica_groups=groups, ins=[a2a_in1.opt()], outs=[a2a_out1.opt()],
            )
            for src in range(NC_CORES):
                nc.sync.dma_start(a2a_sb1[:, src, :], a2a_out1[src, :, :])

            evens = [2 * src for src in range(NC_CORES)]
            odds = [2 * src + 1 for src in range(NC_CORES)]

            def op_mm(psf, qt, nsl, g, start, stop):
                w_ap = (woA[:, g, nsl] if g < DT // 2
                        else woB[:, g - DT // 2, nsl])
                a_ap = (a2a_sb0[:, g // 2, 128 * qt:128 * qt + 128] if g % 2 == 0
                        else a2a_sb1[:, g // 2, 128 * qt:128 * qt + 128])
                nc.tensor.matmul(psf[:], a_ap, w_ap, start=start, stop=stop)

            # pair-0 contributions as CLOSED psum groups saved to SBUF: they
            # only need the first attnT AllToAll, so the PE runs them while
            # the second is in flight (closed groups cannot be reordered
            # behind the pair-1 data). partials reuses proj's dead slot.
            partials = pp.tile([128, 2 * NCH, CH], BF, tag="proj",
                               name="partials")
            chunks = [(qt, nch) for qt in range(2) for nch in range(NCH)]
            for i8, (qt, nch) in enumerate(chunks):
                psf = psp.tile([128, CH], F32, tag="spair", bufs=2, name="psfE")
                nsl = slice(CH * nch, CH * nch + CH)
                for i, g in enumerate(evens):
                    op_mm(psf, qt, nsl, g, i == 0, i == NC_CORES - 1)
                nc.vector.tensor_copy(partials[:, i8, :], psf[:])
            for i8, (qt, nch) in enumerate(chunks):
                psf = psp.tile([128, CH], F32, tag="spair", bufs=2, name="psfO")
                nsl = slice(CH * nch, CH * nch + CH)
                for i, g in enumerate(odds):
                    op_mm(psf, qt, nsl, g, i == 0, i == NC_CORES - 1)
                osb = wp.tile([128, CH], F32, tag="osb", bufs=2, name="osb")
                nc.vector.tensor_tensor(osb[:], psf[:], partials[:, i8, :], ADD)
                nc.sync.dma_start(out[128 * qt:128 * qt + 128, nsl], osb[:])

    nc.finalize()
    return nc


def _get_nc():
    if "nc" not in _CACHE:
        _CACHE["nc"] = _build_nc()
    return _CACHE["nc"]


_PERM = np.concatenate([np.arange(0, HD, 2), np.arange(1, HD, 2)])  # de-interleave


def _shard(inputs):
    x = np.ascontiguousarray(inputs["x"][0].astype(np.float32))          # [S, D]
    wq, wk, wv = (np.asarray(inputs[k]).astype(np.float32) for k in ("wq", "wk", "wv"))
    import ml_dtypes
    wo = np.ascontiguousarray(np.asarray(inputs["wo"]).astype(ml_dtypes.bfloat16))
    cos = np.asarray(inputs["freqs_cos"]).astype(np.float32)
    sin = np.asarray(inputs["freqs_sin"]).astype(np.float32)
    # W_all columns: [q-pair0 (8x128) | q-pair1 (8x128) | k (8x64) | v (8x64)],
    # q/k head-dims de-interleaved ([32 evens | 32 odds] per head)
    wq_p = wq.reshape(DIM, 32, HD)[:, :, _PERM].reshape(DIM, 32, HD)
    wk_p = wk.reshape(DIM, 8, HD)[:, :, _PERM]
    q0 = np.concatenate([wq_p[:, 4 * c:4 * c + 2, :].reshape(DIM, 128)
                         for c in range(NC_CORES)], axis=1)
    q1 = np.concatenate([wq_p[:, 4 * c + 2:4 * c + 4, :].reshape(DIM, 128)
                         for c in range(NC_CORES)], axis=1)
    import ml_dtypes
    w_all = np.ascontiguousarray(
        np.concatenate([q0, q1, wk_p.reshape(DIM, 512), wv], axis=1)
        .astype(ml_dtypes.bfloat16))
    in_maps = []
    for c in range(NC_CORES):
        in_maps.append({
            "x_sl": np.ascontiguousarray(x[SC * c:SC * (c + 1), :]),
            "w_all": w_all,
            "wo": wo,
            "cosR": np.ascontiguousarray(cos[SC * c:SC * (c + 1), :]),
            "sinR": np.ascontiguousarray(sin[SC * c:SC * (c + 1), :]),
        })
    return in_maps


def kernel(**inputs):
    from concourse.bass_utils import run_bass_kernel_spmd

    nc = _get_nc()
    in_maps = _shard(inputs)
    res = run_bass_kernel_spmd(nc, in_maps, core_ids=list(range(NC_CORES)))
    out = np.concatenate([res.results[c]["out"] for c in range(NC_CORES)], axis=0)
    return out[None].astype(np.float32)



# revision 69
# speedup vs baseline: 1.0847x; 1.0509x over previous
"""Tensor-parallel GQA attention forward for one TRN2 chip (8 NeuronCores).

Strategy (8-way tensor parallel over heads):
  - each core owns 4 q-heads + 1 kv-head (wq/wk/wv column-sharded, host side)
  - x is transposed on-device: each core PE-transposes its 256-row slice of x
    (cast to bf16) and an AllGather assembles the full xT on every core
  - projections produce qT/kT (head_dim on partitions) and v (natural layout)
    directly in the layouts the attention matmuls want; RoPE is applied in a
    de-interleaved head-dim ordering (dot products are permutation invariant)
  - scores are computed transposed (S^T[k, q]) so exp runs straight out of
    PSUM; softmax denominators come for free as a 65th column of ones in the
    PV matmul; causal masking = skipping k-tiles above the diagonal plus a
    0/1 pattern multiply on the 4 diagonal-band tiles per chunk
  - an AllToAll flips head-sharded attnT to sequence-sharded, each core then
    computes its 256-row slice of the output projection against full wo
  - compute dtype bf16 (fp32 PSUM accumulation), output fp32
"""

import numpy as np

NC_CORES = 8
SEQ = 2048
DIM = 2048
HD = 64            # head dim
LHEADS = 4         # q heads per core
SC = SEQ // NC_CORES   # 256: sequence rows per core (transpose shard / output shard)
CH = 512           # q-chunk width for attention
NCH = SEQ // CH    # 4
KT = SEQ // 128    # 16 k-tiles
DT = DIM // 128    # 16 d-tiles

_CACHE = {}


def _build_nc():
    import concourse.bass as bass
    import concourse.mybir as mybir
    import concourse.tile as tile
    from concourse import bacc
    from concourse.masks import make_identity

    BF = mybir.dt.bfloat16
    F32 = mybir.dt.float32
    MUL = mybir.AluOpType.mult
    ADD = mybir.AluOpType.add
    SUB = mybir.AluOpType.subtract

    nc = bacc.Bacc("TRN2", target_bir_lowering=False, debug=False,
                   num_devices=NC_CORES)

    # ---- external I/O (per-core shards) ----
    # W_all columns: [q-pair0: 8x128 | q-pair1: 8x128 | k: 8x64 | v: 8x64]
    # xt_sl: this core's 256 seq-columns of x^T, host-pretiled to SBUF layout
    xt_sl = nc.dram_tensor("xt_sl", [128, DT, SC], BF, kind="ExternalInput")
    w_all = nc.dram_tensor("w_all", [DIM, DIM + 2 * 512], BF, kind="ExternalInput")
    wo = nc.dram_tensor("wo", [DIM, DIM], BF, kind="ExternalInput")
    cosR = nc.dram_tensor("cosR", [SC, 32], F32, kind="ExternalInput")
    sinR = nc.dram_tensor("sinR", [SC, 32], F32, kind="ExternalInput")
    out = nc.dram_tensor("out", [SC, DIM], F32, kind="ExternalOutput")

    groups = [list(range(NC_CORES))]
    WCOLS = DIM + 1024          # 3072
    NCH_W = WCOLS // CH         # 6 projection column chunks

    with tile.TileContext(nc) as tc:
        # DRAM bounce buffers for collectives
        apkv_in, _ = tc.tile([NC_CORES, SC, 128], BF, space=bass.MemorySpace.DRAM,
                             name="apkv_in")
        apkv_out, _ = tc.tile([NC_CORES, SC, 128], BF, space=bass.MemorySpace.DRAM,
                              addr_space="Shared", name="apkv_out")
        apq0_in, _ = tc.tile([NC_CORES, SC, 128], BF, space=bass.MemorySpace.DRAM,
                             name="apq0_in")
        apq0_out, _ = tc.tile([NC_CORES, SC, 128], BF, space=bass.MemorySpace.DRAM,
                              addr_space="Shared", name="apq0_out")
        apq1_in, _ = tc.tile([NC_CORES, SC, 128], BF, space=bass.MemorySpace.DRAM,
                             name="apq1_in")
        apq1_out, _ = tc.tile([NC_CORES, SC, 128], BF, space=bass.MemorySpace.DRAM,
                              addr_space="Shared", name="apq1_out")
        a2a_in0, _ = tc.tile([NC_CORES, 128, SC], BF,
                             space=bass.MemorySpace.DRAM, name="a2a_in0")
        a2a_out0, _ = tc.tile([NC_CORES, 128, SC], BF,
                              space=bass.MemorySpace.DRAM,
                              addr_space="Shared", name="a2a_out0")
        a2a_in1, _ = tc.tile([NC_CORES, 128, SC], BF,
                             space=bass.MemorySpace.DRAM, name="a2a_in1")
        a2a_out1, _ = tc.tile([NC_CORES, 128, SC], BF,
                              space=bass.MemorySpace.DRAM,
                              addr_space="Shared", name="a2a_out1")

        with tc.tile_pool(name="persist", bufs=1) as pp, \
             tc.tile_pool(name="wstream", bufs=2) as wsp, \
             tc.tile_pool(name="work", bufs=2) as wp, \
             tc.tile_pool(name="psum", bufs=2, space="PSUM") as psp:

            # ---------------- local transpose of own x slice ----------------
            ident = pp.tile([128, 128], BF, name="ident")
            make_identity(nc, ident[:])

            # prepay the exp ACT-table load (~2.7us) while DMAs stream
            warmup = pp.tile([1, 1], BF, name="warmup")
            nc.scalar.activation(warmup[:], ident[0:1, 0:1],
                                 mybir.ActivationFunctionType.Exp, scale=1.0)

            xTc = pp.tile([128, DT, SC], BF, name="xTc")
            nc.gpsimd.dma_start(xTc[:], xt_sl[:])

            # rope tables, replicated across the 40 roped heads (32 q + 8 k),
            # per local 128-row s-tile
            cosR_sb = pp.tile([128, 2, 32], BF, name="cosR_sb")
            sinR_sb = pp.tile([128, 2, 32], BF, name="sinR_sb")
            nc.gpsimd.dma_start(cosR_sb[:], cosR[:].rearrange("(t p) f -> p t f", p=128))
            nc.gpsimd.dma_start(sinR_sb[:], sinR[:].rearrange("(t p) f -> p t f", p=128))
            cos_rep = pp.tile([128, 2, 8, 32], BF, name="cos_rep")
            sin_rep = pp.tile([128, 2, 8, 32], BF, name="sin_rep")
            for st in range(2):
                for h in range(8):
                    nc.vector.tensor_copy(cos_rep[:, st, h, :], cosR_sb[:, st, :])
                    nc.vector.tensor_copy(sin_rep[:, st, h, :], sinR_sb[:, st, :])

            # ---------------- seq-sharded projections (all heads, own 256 s) ----
            # W chunk order: k, v first (their A2A overlaps the q projections),
            # then q-pair0, then q-pair1; each section's AllToAll is issued as
            # soon as its columns are projected + roped.
            proj = pp.tile([128, 2, WCOLS], BF, name="proj")

            def proj_chunk(ch):
                wt = wsp.tile([128, DT, CH], BF, tag="wt", bufs=2, name="wt")
                for hf in range(2):
                    nc.sync.dma_start(
                        wt[:, 8 * hf:8 * hf + 8, :],
                        w_all[1024 * hf:1024 * hf + 1024, CH * ch:CH * ch + CH]
                        .rearrange("(t p) m -> p t m", p=128))
                for st in range(2):
                    psq = psp.tile([128, CH], F32, tag="ps", bufs=4, name="psq")
                    for dt in range(DT):
                        nc.tensor.matmul(
                            psq[:], xTc[:, dt, 128 * st:128 * st + 128],
                            wt[:, dt, :],
                            start=(dt == 0), stop=(dt == DT - 1))
                    if ch < 5:   # q and k columns get RoPE (8 head-pairs/chunk)
                        nh = 8
                        pv = psq[:].rearrange("p (h x) -> p h x", x=32)
                        ta = wp.tile([128, 8, 32], F32, tag="ropeA", bufs=2, name="ta")
                        tb = wp.tile([128, 8, 32], F32, tag="ropeB", bufs=2, name="tb")
                        dstv = proj[:, st, CH * ch:CH * ch + CH].rearrange(
                            "p (h x) -> p h x", x=32)
                        crep = cos_rep[:, st, 0:nh, :]
                        srep = sin_rep[:, st, 0:nh, :]
                        qr = pv[:, 0:2 * nh:2, :]
                        qi = pv[:, 1:2 * nh:2, :]
                        nc.vector.tensor_tensor(ta[:, 0:nh, :], qr, crep, MUL)
                        nc.vector.tensor_tensor(tb[:, 0:nh, :], qi, srep, MUL)
                        nc.vector.tensor_tensor(dstv[:, 0:2 * nh:2, :],
                                                ta[:, 0:nh, :], tb[:, 0:nh, :], SUB)
                        nc.vector.tensor_tensor(ta[:, 0:nh, :], qr, srep, MUL)
                        nc.vector.tensor_tensor(tb[:, 0:nh, :], qi, crep, MUL)
                        nc.vector.tensor_tensor(dstv[:, 1:2 * nh:2, :],
                                                ta[:, 0:nh, :], tb[:, 0:nh, :], ADD)
                    else:
                        nc.vector.tensor_copy(proj[:, st, CH * ch:CH * ch + CH],
                                              psq[:])

            # --- kv section (stores issued per chunk for an earlier trigger) ---
            proj_chunk(4)
            for dst in range(NC_CORES):
                nc.gpsimd.dma_start(
                    apkv_in[dst, :, 0:64].rearrange("(t p) m -> p t m", p=128),
                    proj[:, :, 2048 + 64 * dst:2048 + 64 * dst + 64])
            proj_chunk(5)
            for dst in range(NC_CORES):
                nc.gpsimd.dma_start(
                    apkv_in[dst, :, 64:128].rearrange("(t p) m -> p t m", p=128),
                    proj[:, :, 2560 + 64 * dst:2560 + 64 * dst + 64])
            nc.gpsimd.collective_compute(
                "AllToAll", mybir.AluOpType.bypass,
                replica_groups=groups, ins=[apkv_in.opt()], outs=[apkv_out.opt()],
            )
            # --- q pair 0 ---
            for ch in (0, 1):
                proj_chunk(ch)
                for dst in range(4 * ch, 4 * ch + 4):
                    nc.gpsimd.dma_start(
                        apq0_in[dst, :, :].rearrange("(t p) m -> p t m", p=128),
                        proj[:, :, 128 * dst:128 * dst + 128])
            nc.gpsimd.collective_compute(
                "AllToAll", mybir.AluOpType.bypass,
                replica_groups=groups, ins=[apq0_in.opt()], outs=[apq0_out.opt()],
            )
            # --- q pair 1 ---
            for ch in (2, 3):
                proj_chunk(ch)
                for dst in range(4 * (ch - 2), 4 * (ch - 2) + 4):
                    nc.gpsimd.dma_start(
                        apq1_in[dst, :, :].rearrange("(t p) m -> p t m", p=128),
                        proj[:, :, 1024 + 128 * dst:1024 + 128 * dst + 128])
            nc.gpsimd.collective_compute(
                "AllToAll", mybir.AluOpType.bypass,
                replica_groups=groups, ins=[apq1_in.opt()], outs=[apq1_out.opt()],
            )

            # ---------------- receiver: build kT / v, then qT per pair ----------
            qT_t = [[pp.tile([128, CH], BF, name=f"qT{p}_{j}")
                     for j in range(NCH)] for p in range(2)]
            kT = pp.tile([128, SEQ], BF, name="kT")
            v_sb = pp.tile([128, KT, 2 * HD], BF, name="v_sb")
            nc.gpsimd.memset(v_sb[:, :, HD:2 * HD], 1.0)

            stage_k = pp.tile([128, KT, 64], BF, name="stage_k")
            for src in range(NC_CORES):
                nc.sync.dma_start(
                    stage_k[:, 2 * src:2 * src + 2, :],
                    apkv_out[src, :, 0:64].rearrange("(t p) m -> p t m", p=128))
                nc.sync.dma_start(
                    v_sb[:, 2 * src:2 * src + 2, 0:HD],
                    apkv_out[src, :, 64:128].rearrange("(t p) m -> p t m", p=128))
            for g in range(KT):
                tk = psp.tile([64, 128], BF, tag="ps", bufs=4, name="tk")
                nc.tensor.transpose(tk[:], stage_k[:, g, :], ident[:])
                nc.vector.tensor_copy(kT[0:64, 128 * g:128 * g + 128], tk[:])
            nc.vector.tensor_copy(kT[64:128, :], kT[0:64, :])

            stage_q = pp.tile([128, 2, KT, 128], BF, name="stage_q")

            def build_qT(pair):
                apq_out = apq0_out if pair == 0 else apq1_out
                for src in range(NC_CORES):
                    nc.sync.dma_start(
                        stage_q[:, pair, 2 * src:2 * src + 2, :],
                        apq_out[src, :, :].rearrange("(t p) m -> p t m", p=128))
                    for st in range(2):
                        g = 2 * src + st
                        tq = psp.tile([128, 128], BF, tag="ps", bufs=4, name="tq")
                        tq_in = stage_q[:, pair, g, :]
                        nc.tensor.transpose(tq[:], tq_in, ident[:])
                        nc.vector.tensor_copy(
                            qT_t[pair][g // 4][:, 128 * (g % 4):128 * (g % 4) + 128],
                            tq[:])

            build_qT(0)

            # causal pattern: one triangle block (q-col >= k-row), both heads
            patd = pp.tile([128, 2, 128], BF, name="patd")
            nc.gpsimd.memset(patd[:], 1.0)
            for half in range(2):
                nc.gpsimd.affine_select(
                    out=patd[:, half, :], in_=patd[:, half, :],
                    compare_op=mybir.AluOpType.is_ge, fill=0.0,
                    base=0, channel_multiplier=-1, pattern=[[1, 128]],
                )

            # ---------------- attention ----------------
            attnT = pp.tile([128, 2, SEQ], BF, name="attnT")

            def attention(pair, j):
                # software-pipelined k-tile loop: scores(t+1) issue while
                # exp(t) runs on ACT, PV(t) follows; scores/exp/PV are all
                # column-trimmed to the causal region of diagonal k-tiles
                nkt = 4 * j + 4
                pso0 = psp.tile([2 * HD, CH], F32, tag="ps", bufs=4, name="pso0")
                pso1 = psp.tile([2 * HD, CH], F32, tag="ps", bufs=4, name="pso1")
                qsl = slice(CH * j, CH * j + CH)
                qTc = qT_t[pair][j]
                pend = []
                for kt in range(nkt):
                    ks = slice(128 * kt, 128 * kt + 128)
                    dt_ = kt - 4 * j
                    off = 128 * dt_ if dt_ >= 0 else 0
                    sp = psp.tile([128, 2, CH], F32, tag="spair", bufs=2,
                                  name="sp")
                    nc.tensor.matmul(sp[:, 0, off:CH], kT[0:64, ks],
                                     qTc[0:64, off:CH], start=True, stop=True)
                    nc.tensor.matmul(sp[:, 1, off:CH], kT[64:128, ks],
                                     qTc[64:128, off:CH], start=True,
                                     stop=True)
                    ep = wp.tile([128, 2, CH], BF, tag="exps", bufs=4,
                                 name="ep")
                    nc.scalar.activation(ep[:, :, off:CH], sp[:, :, off:CH],
                                         mybir.ActivationFunctionType.Exp,
                                         scale=0.125)
                    if dt_ >= 0:
                        nc.vector.tensor_tensor(
                            ep[:, :, off:off + 128], ep[:, :, off:off + 128],
                            patd[:], MUL)
                    for (pkt, pep, poff) in pend:
                        nc.tensor.matmul(pso0[:, poff:CH], v_sb[:, pkt, :],
                                         pep[:, 0, poff:CH],
                                         start=(pkt == 0), stop=False)
                        nc.tensor.matmul(pso1[:, poff:CH], v_sb[:, pkt, :],
                                         pep[:, 1, poff:CH],
                                         start=(pkt == 0), stop=False)
                    pend = [(kt, ep, off)]
                for (pkt, pep, poff) in pend:
                    nc.tensor.matmul(pso0[:, poff:CH], v_sb[:, pkt, :],
                                     pep[:, 0, poff:CH], start=(pkt == 0),
                                     stop=True)
                    nc.tensor.matmul(pso1[:, poff:CH], v_sb[:, pkt, :],
                                     pep[:, 1, poff:CH], start=(pkt == 0),
                                     stop=True)
                for h, pso in ((0, pso0), (1, pso1)):
                    bc = wp.tile([64, CH], F32, tag="bcast", bufs=2, name="bc")
                    nc.vector.tensor_copy(bc[:], pso[HD:2 * HD, :])
                    rc = wp.tile([64, CH], F32, tag="rcp", bufs=2, name="rc")
                    nc.vector.reciprocal_approx_fast(out=rc[:], in_=bc[:])
                    nc.vector.tensor_tensor(
                        attnT[64 * h:64 * h + 64, pair, qsl],
                        pso[0:HD, :], rc[:], MUL)

            woA = pp.tile([128, DT // 2, DIM], BF, name="woA")
            woB = pp.tile([128, DT // 2, DIM], BF, name="woB")
            for j in range(NCH):
                attention(0, j)
                if j == 1:
                    build_qT(1)   # overlaps remaining pair-0 attention
                for dst in (2 * j, 2 * j + 1):
                    nc.gpsimd.dma_start(a2a_in0[dst, :, :],
                                        attnT[:, 0, SC * dst:SC * dst + SC])
                # anchored wo prefetch (the scheduler hoists dep-free DMAs)
                nc.vector.tensor_copy(woA[0:1, 2 * j, 0:1],
                                      attnT[0:1, 0, CH * j:CH * j + 1])
                nc.sync.dma_start(
                    woA[:, 2 * j:2 * j + 2, :],
                    wo[256 * j:256 * j + 256, :].rearrange("(t p) n -> p t n",
                                                           p=128))
            nc.gpsimd.collective_compute(
                "AllToAll", mybir.AluOpType.bypass,
                replica_groups=groups, ins=[a2a_in0.opt()], outs=[a2a_out0.opt()],
            )
            a2a_sb0 = pp.tile([128, NC_CORES, SC], BF, name="a2a_sb0")
            a2a_sb1 = pp.tile([128, NC_CORES, SC], BF, name="a2a_sb1")
            for src in range(NC_CORES):
                nc.sync.dma_start(a2a_sb0[:, src, :], a2a_out0[src, :, :])
            for j in range(NCH):
                attention(1, j)
                for dst in (2 * j, 2 * j + 1):
                    nc.gpsimd.dma_start(a2a_in1[dst, :, :],
                                        attnT[:, 1, SC * dst:SC * dst + SC])
                if j < 2:
                    nc.vector.tensor_copy(woB[0:1, 4 * j, 0:1],
                                          attnT[0:1, 1, CH * j:CH * j + 1])
                    nc.sync.dma_start(
                        woB[:, 4 * j:4 * j + 4, :],
                        wo[1024 + 512 * j:1024 + 512 * j + 512, :]
                        .rearrange("(t p) n -> p t n", p=128))

            # ---------------- final A2A + output projection ----------------
            nc.gpsimd.collective_compute(
                "AllToAll", mybir.AluOpType.bypass,
                replica_groups=groups, ins=[a2a_in1.opt()], outs=[a2a_out1.opt()],
            )
            for src in range(NC_CORES):
                nc.sync.dma_start(a2a_sb1[:, src, :], a2a_out1[src, :, :])

            evens = [2 * src for src in range(NC_CORES)]
            odds = [2 * src + 1 for src in range(NC_CORES)]

            def op_mm(psf, qt, nsl, g, start, stop):
                w_ap = (woA[:, g, nsl] if g < DT // 2
                        else woB[:, g - DT // 2, nsl])
                a_ap = (a2a_sb0[:, g // 2, 128 * qt:128 * qt + 128] if g % 2 == 0
                        else a2a_sb1[:, g // 2, 128 * qt:128 * qt + 128])
                nc.tensor.matmul(psf[:], a_ap, w_ap, start=start, stop=stop)

            # pair-0 contributions as CLOSED psum groups saved to SBUF: they
            # only need the first attnT AllToAll, so the PE runs them while
            # the second is in flight (closed groups cannot be reordered
            # behind the pair-1 data). partials reuses proj's dead slot.
            partials = pp.tile([128, 2 * NCH, CH], BF, tag="proj",
                               name="partials")
            chunks = [(qt, nch) for qt in range(2) for nch in range(NCH)]
            for i8, (qt, nch) in enumerate(chunks):
                psf = psp.tile([128, CH], F32, tag="spair", bufs=2, name="psfE")
                nsl = slice(CH * nch, CH * nch + CH)
                for i, g in enumerate(evens):
                    op_mm(psf, qt, nsl, g, i == 0, i == NC_CORES - 1)
                nc.vector.tensor_copy(partials[:, i8, :], psf[:])
            for i8, (qt, nch) in enumerate(chunks):
                psf = psp.tile([128, CH], F32, tag="spair", bufs=2, name="psfO")
                nsl = slice(CH * nch, CH * nch + CH)
                for i, g in enumerate(odds):
                    op_mm(psf, qt, nsl, g, i == 0, i == NC_CORES - 1)
                osb = wp.tile([128, CH], F32, tag="osb", bufs=2, name="osb")
                nc.vector.tensor_tensor(osb[:], psf[:], partials[:, i8, :], ADD)
                nc.sync.dma_start(out[128 * qt:128 * qt + 128, nsl], osb[:])

    nc.finalize()
    return nc


def _get_nc():
    if "nc" not in _CACHE:
        _CACHE["nc"] = _build_nc()
    return _CACHE["nc"]


_PERM = np.concatenate([np.arange(0, HD, 2), np.arange(1, HD, 2)])  # de-interleave


def _shard(inputs):
    import ml_dtypes
    x = np.ascontiguousarray(inputs["x"][0].astype(np.float32))          # [S, D]
    # per-core slice of x^T, pre-tiled to the SBUF layout [128, DT, SC]
    xT = x.T.astype(ml_dtypes.bfloat16)                                  # [D, S]
    wq, wk, wv = (np.asarray(inputs[k]).astype(np.float32) for k in ("wq", "wk", "wv"))
    wo = np.ascontiguousarray(np.asarray(inputs["wo"]).astype(ml_dtypes.bfloat16))
    cos = np.asarray(inputs["freqs_cos"]).astype(np.float32)
    sin = np.asarray(inputs["freqs_sin"]).astype(np.float32)
    # W_all columns: [q-pair0 (8x128) | q-pair1 (8x128) | k (8x64) | v (8x64)],
    # q/k head-dims de-interleaved ([32 evens | 32 odds] per head)
    wq_p = wq.reshape(DIM, 32, HD)[:, :, _PERM].reshape(DIM, 32, HD)
    wk_p = wk.reshape(DIM, 8, HD)[:, :, _PERM]
    q0 = np.concatenate([wq_p[:, 4 * c:4 * c + 2, :].reshape(DIM, 128)
                         for c in range(NC_CORES)], axis=1)
    q1 = np.concatenate([wq_p[:, 4 * c + 2:4 * c + 4, :].reshape(DIM, 128)
                         for c in range(NC_CORES)], axis=1)
    import ml_dtypes
    w_all = np.ascontiguousarray(
        np.concatenate([q0, q1, wk_p.reshape(DIM, 512), wv], axis=1)
        .astype(ml_dtypes.bfloat16))
    in_maps = []
    for c in range(NC_CORES):
        in_maps.append({
            "xt_sl": np.ascontiguousarray(
                xT[:, SC * c:SC * (c + 1)]
                .reshape(DT, 128, SC).transpose(1, 0, 2)),
            "w_all": w_all,
            "wo": wo,
            "cosR": np.ascontiguousarray(cos[SC * c:SC * (c + 1), :]),
            "sinR": np.ascontiguousarray(sin[SC * c:SC * (c + 1), :]),
        })
    return in_maps


def kernel(**inputs):
    from concourse.bass_utils import run_bass_kernel_spmd

    nc = _get_nc()
    in_maps = _shard(inputs)
    res = run_bass_kernel_spmd(nc, in_maps, core_ids=list(range(NC_CORES)))
    out = np.concatenate([res.results[c]["out"] for c in range(NC_CORES)], axis=0)
    return out[None].astype(np.float32)

